# revision 1
# baseline (speedup 1.0000x reference)
"""GCN layer (improved self-loops) on 8 Trainium2 NeuronCores.

out = D^{-1/2} (A + 2I) D^{-1/2} X W + b,  deg = in_count + 2.

Strategy (SPMD, one program for all 8 cores; only input data differs per core):
  - Nodes sharded by destination: core m owns rows [m*12544, (m+1)*12544).
  - Aggregation BEFORE the matmul: agg[j] = sum_{e->j} n2_e x[src_e] (+ self),
    out = agg @ W + b, with n2_e = dinv[src] dinv[dst] (2 dinv^2 for self).
  - Per-edge gather of x rows via the custom SWDGE dma_gather instruction
    (int16 indices, so the 100352-row fp16 table is split into 4 chunks).
  - Scatter-add via one-hot matmuls on the tensor engine: for each 128-edge
    chunk, S[e, d] = (dloc_e == d) * n2_e, aggT_tile += M_chunk^T ... i.e.
    matmul(lhsT=M [e, feat], rhs=S [e, dst]) accumulates PSUM [feat, dst].
  - Position-static structure: per (tile, src-chunk) edge buckets with
    capacities = round128(max bucket size over the 8 cores), so the single
    SPMD instruction stream is valid for every core; padding slots gather
    row 0 with n2 = 0.
  - Self loops are handled as 128 extra "edges" per tile, loaded from the
    core's own x shard with a plain strided DMA (no gather needed).
  - Normalization n2 computed on device from staged integer degree counts:
    n2 = fac / sqrt((cs+2)(cd+2)), fac in {0 (pad), 1 (edge), 2 (self)}.
  - Final per-tile-pair matmul with W in fp32, bias via ACT, output stored
    transposed [128 feat, nodes]; host transposes back.
"""

import sys

sys.path.insert(0, "/opt/trn_rl_repo")

import numpy as np

import concourse.bacc as bacc
import concourse.mybir as mybir
import concourse.tile as tile

F32 = mybir.dt.float32
F16 = mybir.dt.float16
I16 = mybir.dt.int16

N = 100000
FEAT = 128
NCORES = 8
PC = 12544            # nodes per core
NPAD = PC * NCORES    # 100352
TILES = PC // 128     # 98
NCHUNK = 4
CHUNK = NPAD // NCHUNK  # 25088 rows per gather chunk
VT = 8                # tiles per compute wave (PSUM pairs)
GVT = 32              # tiles per gather-call group
ACT_EVERY = 4         # every ACT_EVERY'th S-build goes to the scalar engine


def _round128(x):
    return int(-(-int(x) // 128) * 128)


def build_plan(src, dst, cnt):
    """Host-side integer metadata. src/dst: int64 [E]; cnt: int64 [N] in-degree.

    Bucket capacities are the max bucket size over the 8 cores rounded to 16
    (not 128), so almost no padding rows are gathered. 128-edge matmul chunks
    may straddle two adjacent buckets (= two tiles); the parity of a bucket's
    ordinal within its call is encoded into dloc (+128 for odd) so the two
    one-hot matmuls of a straddling chunk can address their own tile.
    """
    E = src.shape[0]
    core = dst // PC
    dl = dst - core * PC          # 0..PC-1
    t = dl >> 7                   # tile in core
    c = src // CHUNK              # source chunk
    flat = (core * TILES + t) * NCHUNK + c
    bc = np.bincount(flat, minlength=NCORES * TILES * NCHUNK).reshape(
        NCORES, TILES, NCHUNK
    )
    B = bc.max(axis=0)            # [TILES, NCHUNK]
    B = ((B + 15) // 16) * 16     # capacities, multiple of 16 (0 stays 0)

    waves = [list(range(w * VT, min((w + 1) * VT, TILES))) for w in range((TILES + VT - 1) // VT)]
    gwaves = [list(range(g * GVT, min((g + 1) * GVT, TILES))) for g in range((TILES + GVT - 1) // GVT)]

    # --- slot layout (same for every core) ---
    # per wave: [call c=0][call c=1][call c=2][call c=3][self region]
    bucket_base = np.zeros((TILES, NCHUNK), np.int64)  # global slot base
    parity = np.zeros((TILES, NCHUNK), np.int64)
    call_nidx = []   # [wave][chunk] -> num idxs (mult of 128)
    call_slot = []   # [wave][chunk] -> slot base
    call_g16 = []    # [wave][chunk] -> eidx col16 base
    chunk_mms = []   # [wave][chunk] -> list per 128-chunk of [(tile, side)]
    self_slot = []   # [wave] -> slot base of self region
    pos = 0
    g16 = 0
    for wave in gwaves:
        nidx_w, slot_w, g_w, mm_w = [], [], [], []
        for cc in range(NCHUNK):
            nonempty = [tt for tt in wave if B[tt, cc] > 0]
            raw = int(sum(B[tt, cc] for tt in nonempty))
            nidx = _round128(raw)
            slot_w.append(pos)
            g_w.append(g16)
            nidx_w.append(nidx)
            # bucket spans (call-local) and parity
            spans = []
            off = 0
            for o, tt in enumerate(nonempty):
                bucket_base[tt, cc] = pos + off
                parity[tt, cc] = o % 2
                spans.append((off, off + int(bc[:, tt, cc].max()), tt, o % 2))
                off += int(B[tt, cc])
            # per-128-chunk real-bucket intersections
            mms = []
            for j in range(nidx // 128):
                lo, hi = j * 128, (j + 1) * 128
                hit = [(tt, par) for (s0, s1, tt, par) in spans
                       if not (s1 <= lo or s0 >= hi)]
                assert len(hit) <= 2, (len(hit), j, cc)
                mms.append(hit)
            mm_w.append(mms)
            pos += nidx
            g16 += nidx // 16
        call_nidx.append(nidx_w)
        call_slot.append(slot_w)
        call_g16.append(g_w)
        chunk_mms.append(mm_w)
        self_slot.append(pos)
        pos += len(wave) * 128
    total_slots = pos
    gcols16 = g16
    cols = total_slots // 128

    # --- per-core arrays ---
    cnt_pad = np.zeros(NPAD, np.int64)
    cnt_pad[:N] = cnt
    eidx_flat = np.zeros((NCORES, gcols16 * 16), np.int16)
    dloc_flat = np.zeros((NCORES, total_slots), np.float16)
    cs_flat = np.zeros((NCORES, total_slots), np.float16)
    cd_flat = np.zeros((NCORES, total_slots), np.float16)
    fac_flat = np.zeros((NCORES, total_slots), np.float16)

    is_self = np.zeros(total_slots, bool)
    for w, wave in enumerate(gwaves):
        is_self[self_slot[w] : self_slot[w] + len(wave) * 128] = True
    g_of_slot = np.cumsum(~is_self) - 1

    order_all = np.argsort(core * (TILES * NCHUNK) + t * NCHUNK + c, kind="stable")
    flat_sorted = flat[order_all]
    starts = np.searchsorted(flat_sorted, np.arange(NCORES * TILES * NCHUNK), side="left")
    rank = np.arange(E) - starts[flat_sorted]
    bb_flat = np.broadcast_to(bucket_base, (NCORES, TILES, NCHUNK)).reshape(-1)
    par_flat = np.broadcast_to(parity, (NCORES, TILES, NCHUNK)).reshape(-1)
    slots_sorted = bb_flat[flat_sorted] + rank
    par_sorted = par_flat[flat_sorted]
    cores_sorted = core[order_all]
    src_sorted = src[order_all]
    dst_sorted = dst[order_all]
    c_sorted = c[order_all]
    for m in range(NCORES):
        sel = cores_sorted == m
        sl = slots_sorted[sel]
        eidx_flat[m, g_of_slot[sl]] = (src_sorted[sel] - c_sorted[sel] * CHUNK).astype(np.int16)
        dloc_flat[m, sl] = ((dst_sorted[sel] & 127) + 128 * par_sorted[sel]).astype(np.float16)
        cs_flat[m, sl] = cnt_pad[src_sorted[sel]].astype(np.float16)
        cd_flat[m, sl] = cnt_pad[dst_sorted[sel]].astype(np.float16)
        fac_flat[m, sl] = 1.0

    for w, wave in enumerate(gwaves):
        nsw = len(wave) * 128
        sl = self_slot[w] + np.arange(nsw)
        nodes_l = wave[0] * 128 + np.arange(nsw)
        for m in range(NCORES):
            nodes_g = m * PC + nodes_l
            real = nodes_g < N
            dloc_flat[m, sl] = (nodes_l & 127).astype(np.float16)
            cs_flat[m, sl] = cnt_pad[np.minimum(nodes_g, N - 1)].astype(np.float16) * real
            cd_flat[m, sl] = cs_flat[m, sl]
            fac_flat[m, sl] = np.where(real, np.float16(2.0), np.float16(0.0))

    def wrap(a):
        return np.ascontiguousarray(a.reshape(-1, 128).T)

    eidx = np.zeros((NCORES, 128, gcols16), np.int16)
    for m in range(NCORES):
        w16 = eidx_flat[m].reshape(-1, 16).T
        eidx[m] = np.tile(w16, (8, 1))

    return dict(
        B=B, waves=waves, gwaves=gwaves, call_nidx=call_nidx, call_slot=call_slot,
        call_g16=call_g16, self_slot=self_slot, total_slots=total_slots,
        gcols16=gcols16, cols=cols, bucket_base=bucket_base,
        chunk_mms=chunk_mms,
        eidx=eidx,
        dloc=np.stack([wrap(dloc_flat[m]) for m in range(NCORES)]),
        cs=np.stack([wrap(cs_flat[m]) for m in range(NCORES)]),
        cd=np.stack([wrap(cd_flat[m]) for m in range(NCORES)]),
        fac=np.stack([wrap(fac_flat[m]) for m in range(NCORES)]),
    )


def build_bass(plan, repeat=1, mode="full", qspread=False):
    """Build the SPMD Bass program for the static structure in `plan`."""
    B = plan["B"]
    waves = plan["waves"]
    cols = plan["cols"]
    gcols16 = plan["gcols16"]

    nc = bacc.Bacc("TRN2", target_bir_lowering=False, debug=False)
    xt = nc.dram_tensor("xt", [NPAD, FEAT], F16, kind="ExternalInput")
    xself = nc.dram_tensor("xself", [PC, FEAT], F16, kind="ExternalInput")
    eidx_d = nc.dram_tensor("eidx", [128, gcols16], I16, kind="ExternalInput")
    dloc_d = nc.dram_tensor("dloc", [128, cols], F16, kind="ExternalInput")
    cs_d = nc.dram_tensor("cs", [128, cols], F16, kind="ExternalInput")
    cd_d = nc.dram_tensor("cd", [128, cols], F16, kind="ExternalInput")
    fac_d = nc.dram_tensor("fac", [128, cols], F16, kind="ExternalInput")
    w_d = nc.dram_tensor("w", [FEAT, FEAT], F32, kind="ExternalInput")
    bcol_d = nc.dram_tensor("bcol", [FEAT, 1], F32, kind="ExternalInput")
    iota_d = nc.dram_tensor("iota", [128, 256], F16, kind="ExternalInput")
    outT = nc.dram_tensor("outT", [FEAT, PC], F32, kind="ExternalOutput")

    with tile.TileContext(nc) as tc:
        with (
            tc.tile_pool(name="meta", bufs=1) as meta,
            tc.tile_pool(name="mg", bufs=6) as mgp,
            tc.tile_pool(name="ms", bufs=4) as msp,
            tc.tile_pool(name="sp", bufs=16) as spool,
            tc.tile_pool(name="fin", bufs=6) as fin,
            tc.tile_pool(name="aggps", bufs=6, space="PSUM") as aggps,
            tc.tile_pool(name="outps", bufs=2, space="PSUM") as outps,
        ):
            # ---- prologue: metadata loads + bulk normalization ----
            sb_eidx = meta.tile([128, gcols16], I16, tag="eidx")
            nc.sync.dma_start(sb_eidx[:], eidx_d[:])
            sb_dloch = meta.tile([128, cols], F16, tag="dloch")
            nc.sync.dma_start(sb_dloch[:], dloc_d[:])
            sb_cs = meta.tile([128, cols], F16, tag="csh")
            nc.sync.dma_start(sb_cs[:], cs_d[:])
            sb_cd = meta.tile([128, cols], F16, tag="cdh")
            nc.sync.dma_start(sb_cd[:], cd_d[:])
            sb_fac = meta.tile([128, cols], F16, tag="fach")
            nc.sync.dma_start(sb_fac[:], fac_d[:])
            sb_w = meta.tile([FEAT, FEAT], F32, tag="w")
            nc.sync.dma_start(sb_w[:], w_d[:])
            sb_bcol = meta.tile([FEAT, 1], F32, tag="bcol")
            nc.sync.dma_start(sb_bcol[:], bcol_d[:])
            sb_iota = meta.tile([128, 256], F16, tag="iota")
            nc.sync.dma_start(sb_iota[:], iota_d[:])

            sb_dloc = meta.tile([128, cols], F32, tag="dlocf")
            nc.vector.tensor_copy(sb_dloc[:], sb_dloch[:])
            sb_t1 = meta.tile([128, cols], F32, tag="t1")
            nc.vector.tensor_scalar_add(sb_t1[:], sb_cs[:], 2.0)
            sb_t2 = meta.tile([128, cols], F32, tag="t2")
            nc.vector.tensor_scalar_add(sb_t2[:], sb_cd[:], 2.0)
            nc.vector.tensor_mul(sb_t1[:], sb_t1[:], sb_t2[:])
            nc.scalar.activation(sb_t2[:], sb_t1[:], mybir.ActivationFunctionType.Sqrt)
            nc.vector.reciprocal(sb_t1[:], sb_t2[:])
            sb_facf = meta.tile([128, cols], F32, tag="facf")
            nc.vector.tensor_copy(sb_facf[:], sb_fac[:])
            sb_n2 = meta.tile([128, cols], F32, tag="n2")
            nc.vector.tensor_mul(sb_n2[:], sb_t1[:], sb_facf[:])
            sb_n2n = meta.tile([128, cols], F32, tag="n2n")
            nc.vector.tensor_scalar_mul(sb_n2n[:], sb_n2[:], -1.0)

            sb_count = [0]
            import contextlib
            loop_cm = tc.For_i(0, repeat, 1) if repeat > 1 else contextlib.nullcontext()

            def build_s(gcol, side=0):
                """S[e, d] = (dloc[e] == side*128 + d) * n2[e] for the
                128-edge chunk at global column gcol. Alternates DVE / ACT."""
                sb_count[0] += 1
                io = sb_iota[:, side * 128 : side * 128 + 128]
                if sb_count[0] % ACT_EVERY == 0:
                    ta = spool.tile([128, 128], F16, tag="sa")
                    nc.scalar.activation(
                        ta[:], io, mybir.ActivationFunctionType.Abs,
                        bias=sb_dloc[:, gcol : gcol + 1], scale=-1.0,
                    )
                    s = spool.tile([128, 128], F16, tag="sb")
                    nc.scalar.activation(
                        s[:], ta[:], mybir.ActivationFunctionType.Relu,
                        bias=sb_n2[:, gcol : gcol + 1],
                        scale=sb_n2n[:, gcol : gcol + 1],
                    )
                else:
                    s = spool.tile([128, 128], F16, tag="sb")
                    nc.vector.tensor_scalar(
                        s[:], io,
                        sb_dloc[:, gcol : gcol + 1], sb_n2[:, gcol : gcol + 1],
                        mybir.AluOpType.is_equal, mybir.AluOpType.mult,
                    )
                return s

            # ---- main loop: gather groups (gwaves) / compute waves ----
            with loop_cm:
              for g, gtiles in enumerate(plan["gwaves"]):
                  mtiles = {}
                  for cc in range(NCHUNK):
                      nidx = plan["call_nidx"][g][cc]
                      if nidx == 0:
                          continue
                      m = mgp.tile([128, nidx // 128, 128], F16, tag="mg")
                      g16 = plan["call_g16"][g][cc]
                      nc.gpsimd.dma_gather(
                          m[:, : nidx // 128, :],
                          xt[cc * CHUNK : (cc + 1) * CHUNK, :],
                          sb_eidx[:, g16 : g16 + nidx // 16],
                          nidx, nidx, FEAT,
                          single_packet=(nidx <= 1024),
                      )
                      mtiles[cc] = m
                  nsw = len(gtiles)
                  ms = msp.tile([128, nsw, 128], F16, tag="ms")
                  r0 = gtiles[0] * 128
                  nc.sync.dma_start(
                      ms[:], xself[r0 : r0 + nsw * 128, :].rearrange("(n p) d -> p n d", p=128)
                  )

                  if mode == "gather":
                      o = fin.tile([128, 128], F32, tag="gonly", name="gonly")
                      nc.vector.tensor_copy(o[:], ms[:, 0, :])
                      nc.sync.dma_start(outT[:, gtiles[0] * 128 : gtiles[0] * 128 + 128], o[:])
                      for cc in range(NCHUNK):
                          if cc in mtiles:
                              o2 = fin.tile([128, 128], F32, tag="gonly2", name="gonly2")
                              nc.vector.tensor_copy(o2[:], mtiles[cc][:, 0, :])
                              nc.sync.dma_start(outT[:, gtiles[0] * 128 : gtiles[0] * 128 + 128], o2[:])
                      continue

                  per_tile = {t: [] for t in gtiles}
                  for cc in range(NCHUNK):
                      if plan["call_nidx"][g][cc] == 0:
                          continue
                      call_base = plan["call_slot"][g][cc]
                      for j, hits in enumerate(plan["chunk_mms"][g][cc]):
                          gcol = (call_base + j * 128) // 128
                          for (t, side) in hits:
                              per_tile[t].append((cc, j, gcol, side))

                  for cw0 in range(0, len(gtiles), VT):
                      cwave = gtiles[cw0 : cw0 + VT]
                      pairs = {}
                      started = {}

                      def pair_half(t, cwave=cwave, pairs=pairs):
                          ti = t - cwave[0]
                          pi = ti // 2
                          if pi not in pairs:
                              pairs[pi] = aggps.tile([128, 256], F32, tag="agg", name="agg")
                          return pairs[pi], (ti % 2) * 128

                      for t in cwave:
                          for (cc, j, gcol, side) in per_tile[t]:
                              ppair, half = pair_half(t)
                              s = build_s(gcol, side)
                              nc.tensor.matmul(
                                  ppair[:, half : half + 128],
                                  mtiles[cc][:, j, :],
                                  s[:],
                                  start=(t not in started), stop=False,
                                  skip_group_check=True,
                              )
                              started[t] = True
                          ppair, half = pair_half(t)
                          tig = t - gtiles[0]
                          gcol = (plan["self_slot"][g] + tig * 128) // 128
                          s = build_s(gcol, 0)
                          nc.tensor.matmul(
                              ppair[:, half : half + 128], ms[:, tig, :], s[:],
                              start=(t not in started), stop=True, skip_group_check=True,
                          )

                      for pi in sorted(pairs):
                          ppair = pairs[pi]
                          asb = fin.tile([128, 256], F32, tag="asb")
                          nc.vector.tensor_copy(asb[:], ppair[:])
                          op = outps.tile([128, 256], F32, tag="op")
                          nc.tensor.matmul(op[:], sb_w[:], asb[:], skip_group_check=True)
                          osb = fin.tile([128, 256], F32, tag="osb")
                          nc.scalar.activation(
                              osb[:], op[:], mybir.ActivationFunctionType.Identity,
                              bias=sb_bcol[:],
                          )
                          base = (cwave[0] + pi * 2) * 128
                          nc.sync.dma_start(outT[:, base : base + 256], osb[:])
    nc.compile()
    return nc


_CACHE = {}


def _get_compiled(src, dst, cnt):
    plan = build_plan(src, dst, cnt)
    key = (plan["total_slots"], plan["B"].tobytes())
    if key not in _CACHE:
        _CACHE[key] = (build_bass(plan), plan)
    else:
        _CACHE[key] = (_CACHE[key][0], plan)
    return _CACHE[key]


def kernel(x, edge_index, W, b):
    from concourse.bass_utils import run_bass_kernel_spmd

    x = np.asarray(x)
    edge_index = np.asarray(edge_index)
    W = np.asarray(W)
    b = np.asarray(b)
    src = edge_index[0].astype(np.int64)
    dst = edge_index[1].astype(np.int64)
    cnt = np.bincount(dst, minlength=N)

    nc, plan = _get_compiled(src, dst, cnt)

    xt = np.zeros((NPAD, FEAT), np.float16)
    xt[:N] = x.astype(np.float16)
    iota = np.tile(np.arange(256, dtype=np.float16), (128, 1))
    wf = W.astype(np.float32)
    bcol = b.astype(np.float32).reshape(FEAT, 1)

    in_maps = []
    for m in range(NCORES):
        in_maps.append({
            "xt": xt,
            "xself": xt[m * PC : (m + 1) * PC],
            "eidx": plan["eidx"][m],
            "dloc": plan["dloc"][m],
            "cs": plan["cs"][m],
            "cd": plan["cd"][m],
            "fac": plan["fac"][m],
            "w": wf,
            "bcol": bcol,
            "iota": iota,
        })
    res = run_bass_kernel_spmd(nc, in_maps, list(range(NCORES)))
    outT = np.concatenate([res.results[m]["outT"] for m in range(NCORES)], axis=1)
    return np.ascontiguousarray(outT[:, :N].T).astype(np.float32)



# revision 3
# speedup vs baseline: 1.7518x; 1.7518x over previous
"""GCN layer (improved self-loops) on 8 Trainium2 NeuronCores.

out = D^{-1/2} (A + 2I) D^{-1/2} X W + b,  deg = in_count + 2.

Strategy (SPMD, one program for all 8 cores; only input data differs per core):
  - Nodes sharded by destination: core m owns rows [m*12544, (m+1)*12544).
  - Aggregation BEFORE the matmul: agg[j] = sum_{e->j} n2_e x[src_e] (+ self),
    out = agg @ W + b, with n2_e = dinv[src] dinv[dst] (2 dinv^2 for self).
  - Per-edge gather of x rows via the custom SWDGE dma_gather instruction
    (int16 indices, so the 100352-row fp16 table is split into 4 chunks).
  - Scatter-add via one-hot matmuls on the tensor engine: for each 128-edge
    chunk, S[e, d] = (dloc_e == d) * n2_e, aggT_tile += M_chunk^T ... i.e.
    matmul(lhsT=M [e, feat], rhs=S [e, dst]) accumulates PSUM [feat, dst].
  - Position-static structure: per (tile, src-chunk) edge buckets with
    capacities = round128(max bucket size over the 8 cores), so the single
    SPMD instruction stream is valid for every core; padding slots gather
    row 0 with n2 = 0.
  - Self loops are handled as 128 extra "edges" per tile, loaded from the
    core's own x shard with a plain strided DMA (no gather needed).
  - Normalization n2 computed on device from staged integer degree counts:
    n2 = fac / sqrt((cs+2)(cd+2)), fac in {0 (pad), 1 (edge), 2 (self)}.
  - Final per-tile-pair matmul with W in fp32, bias via ACT, output stored
    transposed [128 feat, nodes]; host transposes back.
"""

import sys

sys.path.insert(0, "/opt/trn_rl_repo")

import numpy as np

import concourse.bacc as bacc
import concourse.mybir as mybir
import concourse.tile as tile

F32 = mybir.dt.float32
F16 = mybir.dt.float16
I16 = mybir.dt.int16

N = 100000
FEAT = 128
NCORES = 8
PC = 12544            # nodes per core
NPAD = PC * NCORES    # 100352
TILES = PC // 128     # 98
NCHUNK = 4
CHUNK = NPAD // NCHUNK  # 25088 rows per gather chunk
VT = 8                # tiles per compute wave (PSUM pairs)
GVT = 32              # tiles per gather-call group
ACT_EVERY = 4         # every ACT_EVERY'th S-build goes to the scalar engine


def _round128(x):
    return int(-(-int(x) // 128) * 128)


def build_plan(src, dst, cnt):
    """Host-side integer metadata. src/dst: int64 [E]; cnt: int64 [N] in-degree.

    Bucket capacities are the max bucket size over the 8 cores rounded to 16
    (not 128), so almost no padding rows are gathered. 128-edge matmul chunks
    may straddle two adjacent buckets (= two tiles); the parity of a bucket's
    ordinal within its call is encoded into dloc (+128 for odd) so the two
    one-hot matmuls of a straddling chunk can address their own tile.
    """
    E = src.shape[0]
    core = dst // PC
    dl = dst - core * PC          # 0..PC-1
    t = dl >> 7                   # tile in core
    c = src // CHUNK              # source chunk
    flat = (core * TILES + t) * NCHUNK + c
    bc = np.bincount(flat, minlength=NCORES * TILES * NCHUNK).reshape(
        NCORES, TILES, NCHUNK
    )
    B = bc.max(axis=0)            # [TILES, NCHUNK]
    B = ((B + 15) // 16) * 16     # capacities, multiple of 16 (0 stays 0)

    waves = [list(range(w * VT, min((w + 1) * VT, TILES))) for w in range((TILES + VT - 1) // VT)]
    gwaves = [list(range(g * GVT, min((g + 1) * GVT, TILES))) for g in range((TILES + GVT - 1) // GVT)]

    # --- slot layout (same for every core) ---
    # per wave: [call c=0][call c=1][call c=2][call c=3][self region]
    bucket_base = np.zeros((TILES, NCHUNK), np.int64)  # global slot base
    parity = np.zeros((TILES, NCHUNK), np.int64)
    call_nidx = []   # [wave][chunk] -> num idxs (mult of 128)
    call_slot = []   # [wave][chunk] -> slot base
    call_g16 = []    # [wave][chunk] -> eidx col16 base
    chunk_mms = []   # [wave][chunk] -> list per 128-chunk of [(tile, side)]
    self_slot = []   # [wave] -> slot base of self region
    pos = 0
    g16 = 0
    for wave in gwaves:
        nidx_w, slot_w, g_w, mm_w = [], [], [], []
        for cc in range(NCHUNK):
            nonempty = [tt for tt in wave if B[tt, cc] > 0]
            raw = int(sum(B[tt, cc] for tt in nonempty))
            nidx = _round128(raw)
            slot_w.append(pos)
            g_w.append(g16)
            nidx_w.append(nidx)
            # bucket spans (call-local) and parity
            spans = []
            off = 0
            for o, tt in enumerate(nonempty):
                bucket_base[tt, cc] = pos + off
                parity[tt, cc] = o % 2
                spans.append((off, off + int(bc[:, tt, cc].max()), tt, o % 2))
                off += int(B[tt, cc])
            # per-128-chunk real-bucket intersections
            mms = []
            for j in range(nidx // 128):
                lo, hi = j * 128, (j + 1) * 128
                hit = [(tt, par) for (s0, s1, tt, par) in spans
                       if not (s1 <= lo or s0 >= hi)]
                assert len(hit) <= 2, (len(hit), j, cc)
                mms.append(hit)
            mm_w.append(mms)
            pos += nidx
            g16 += nidx // 16
        call_nidx.append(nidx_w)
        call_slot.append(slot_w)
        call_g16.append(g_w)
        chunk_mms.append(mm_w)
        self_slot.append(pos)
        pos += len(wave) * 128
    total_slots = pos
    gcols16 = g16
    cols = total_slots // 128

    # --- per-core arrays ---
    cnt_pad = np.zeros(NPAD, np.int64)
    cnt_pad[:N] = cnt
    eidx_flat = np.zeros((NCORES, gcols16 * 16), np.int16)
    dloc_flat = np.zeros((NCORES, total_slots), np.float16)
    cs_flat = np.zeros((NCORES, total_slots), np.float16)
    cd_flat = np.zeros((NCORES, total_slots), np.float16)
    fac_flat = np.zeros((NCORES, total_slots), np.float16)

    is_self = np.zeros(total_slots, bool)
    for w, wave in enumerate(gwaves):
        is_self[self_slot[w] : self_slot[w] + len(wave) * 128] = True
    g_of_slot = np.cumsum(~is_self) - 1

    order_all = np.argsort(core * (TILES * NCHUNK) + t * NCHUNK + c, kind="stable")
    flat_sorted = flat[order_all]
    starts = np.searchsorted(flat_sorted, np.arange(NCORES * TILES * NCHUNK), side="left")
    rank = np.arange(E) - starts[flat_sorted]
    bb_flat = np.broadcast_to(bucket_base, (NCORES, TILES, NCHUNK)).reshape(-1)
    par_flat = np.broadcast_to(parity, (NCORES, TILES, NCHUNK)).reshape(-1)
    slots_sorted = bb_flat[flat_sorted] + rank
    par_sorted = par_flat[flat_sorted]
    cores_sorted = core[order_all]
    src_sorted = src[order_all]
    dst_sorted = dst[order_all]
    c_sorted = c[order_all]
    for m in range(NCORES):
        sel = cores_sorted == m
        sl = slots_sorted[sel]
        eidx_flat[m, g_of_slot[sl]] = (src_sorted[sel] - c_sorted[sel] * CHUNK).astype(np.int16)
        dloc_flat[m, sl] = ((dst_sorted[sel] & 127) + 128 * par_sorted[sel]).astype(np.float16)
        cs_flat[m, sl] = cnt_pad[src_sorted[sel]].astype(np.float16)
        cd_flat[m, sl] = cnt_pad[dst_sorted[sel]].astype(np.float16)
        fac_flat[m, sl] = 1.0

    for w, wave in enumerate(gwaves):
        nsw = len(wave) * 128
        sl = self_slot[w] + np.arange(nsw)
        nodes_l = wave[0] * 128 + np.arange(nsw)
        for m in range(NCORES):
            nodes_g = m * PC + nodes_l
            real = nodes_g < N
            dloc_flat[m, sl] = (nodes_l & 127).astype(np.float16)
            cs_flat[m, sl] = cnt_pad[np.minimum(nodes_g, N - 1)].astype(np.float16) * real
            cd_flat[m, sl] = cs_flat[m, sl]
            fac_flat[m, sl] = np.where(real, np.float16(2.0), np.float16(0.0))

    def wrap(a):
        return np.ascontiguousarray(a.reshape(-1, 128).T)

    eidx = np.zeros((NCORES, 128, gcols16), np.int16)
    for m in range(NCORES):
        w16 = eidx_flat[m].reshape(-1, 16).T
        eidx[m] = np.tile(w16, (8, 1))

    return dict(
        B=B, waves=waves, gwaves=gwaves, call_nidx=call_nidx, call_slot=call_slot,
        call_g16=call_g16, self_slot=self_slot, total_slots=total_slots,
        gcols16=gcols16, cols=cols, bucket_base=bucket_base,
        chunk_mms=chunk_mms,
        eidx=eidx,
        dloc=np.stack([wrap(dloc_flat[m]) for m in range(NCORES)]),
        cs=np.stack([wrap(cs_flat[m]) for m in range(NCORES)]),
        cd=np.stack([wrap(cd_flat[m]) for m in range(NCORES)]),
        fac=np.stack([wrap(fac_flat[m]) for m in range(NCORES)]),
    )


def build_bass(plan, repeat=1, mode="full", qspread=False):
    """Build the SPMD Bass program for the static structure in `plan`."""
    B = plan["B"]
    waves = plan["waves"]
    cols = plan["cols"]
    gcols16 = plan["gcols16"]

    nc = bacc.Bacc(
        "TRN2", target_bir_lowering=False, debug=False, num_swdge_queues=4
    )
    xt = nc.dram_tensor("xt", [NPAD, FEAT], F16, kind="ExternalInput")
    xself = nc.dram_tensor("xself", [PC, FEAT], F16, kind="ExternalInput")
    eidx_d = nc.dram_tensor("eidx", [128, gcols16], I16, kind="ExternalInput")
    dloc_d = nc.dram_tensor("dloc", [128, cols], F16, kind="ExternalInput")
    cs_d = nc.dram_tensor("cs", [128, cols], F16, kind="ExternalInput")
    cd_d = nc.dram_tensor("cd", [128, cols], F16, kind="ExternalInput")
    fac_d = nc.dram_tensor("fac", [128, cols], F16, kind="ExternalInput")
    w_d = nc.dram_tensor("w", [FEAT, FEAT], F32, kind="ExternalInput")
    bcol_d = nc.dram_tensor("bcol", [FEAT, 1], F32, kind="ExternalInput")
    iota_d = nc.dram_tensor("iota", [128, 256], F16, kind="ExternalInput")
    outT = nc.dram_tensor("outT", [FEAT, PC], F32, kind="ExternalOutput")

    with tile.TileContext(nc) as tc:
        with (
            tc.tile_pool(name="meta", bufs=1) as meta,
            tc.tile_pool(name="mg", bufs=6) as mgp,
            tc.tile_pool(name="ms", bufs=4) as msp,
            tc.tile_pool(name="sp", bufs=16) as spool,
            tc.tile_pool(name="fin", bufs=6) as fin,
            tc.tile_pool(name="aggps", bufs=6, space="PSUM") as aggps,
            tc.tile_pool(name="outps", bufs=2, space="PSUM") as outps,
        ):
            # ---- prologue: metadata loads + bulk normalization ----
            sb_eidx = meta.tile([128, gcols16], I16, tag="eidx")
            nc.sync.dma_start(sb_eidx[:], eidx_d[:])
            sb_dloch = meta.tile([128, cols], F16, tag="dloch")
            nc.sync.dma_start(sb_dloch[:], dloc_d[:])
            sb_cs = meta.tile([128, cols], F16, tag="csh")
            nc.sync.dma_start(sb_cs[:], cs_d[:])
            sb_cd = meta.tile([128, cols], F16, tag="cdh")
            nc.sync.dma_start(sb_cd[:], cd_d[:])
            sb_fac = meta.tile([128, cols], F16, tag="fach")
            nc.sync.dma_start(sb_fac[:], fac_d[:])
            sb_w = meta.tile([FEAT, FEAT], F32, tag="w")
            nc.sync.dma_start(sb_w[:], w_d[:])
            sb_bcol = meta.tile([FEAT, 1], F32, tag="bcol")
            nc.sync.dma_start(sb_bcol[:], bcol_d[:])
            sb_iota = meta.tile([128, 256], F16, tag="iota")
            nc.sync.dma_start(sb_iota[:], iota_d[:])

            sb_dloc = meta.tile([128, cols], F32, tag="dlocf")
            nc.vector.tensor_copy(sb_dloc[:], sb_dloch[:])
            sb_t1 = meta.tile([128, cols], F32, tag="t1")
            nc.vector.tensor_scalar_add(sb_t1[:], sb_cs[:], 2.0)
            sb_t2 = meta.tile([128, cols], F32, tag="t2")
            nc.vector.tensor_scalar_add(sb_t2[:], sb_cd[:], 2.0)
            nc.vector.tensor_mul(sb_t1[:], sb_t1[:], sb_t2[:])
            nc.scalar.activation(sb_t2[:], sb_t1[:], mybir.ActivationFunctionType.Sqrt)
            nc.vector.reciprocal(sb_t1[:], sb_t2[:])
            sb_facf = meta.tile([128, cols], F32, tag="facf")
            nc.vector.tensor_copy(sb_facf[:], sb_fac[:])
            sb_n2 = meta.tile([128, cols], F32, tag="n2")
            nc.vector.tensor_mul(sb_n2[:], sb_t1[:], sb_facf[:])
            sb_n2n = meta.tile([128, cols], F32, tag="n2n")
            nc.vector.tensor_scalar_mul(sb_n2n[:], sb_n2[:], -1.0)

            sb_count = [0]
            import contextlib
            loop_cm = tc.For_i(0, repeat, 1) if repeat > 1 else contextlib.nullcontext()

            def build_s(gcol, side=0):
                """S[e, d] = (dloc[e] == side*128 + d) * n2[e] for the
                128-edge chunk at global column gcol. Alternates DVE / ACT."""
                sb_count[0] += 1
                io = sb_iota[:, side * 128 : side * 128 + 128]
                if sb_count[0] % ACT_EVERY == 0:
                    ta = spool.tile([128, 128], F16, tag="sa")
                    nc.scalar.activation(
                        ta[:], io, mybir.ActivationFunctionType.Abs,
                        bias=sb_dloc[:, gcol : gcol + 1], scale=-1.0,
                    )
                    s = spool.tile([128, 128], F16, tag="sb")
                    nc.scalar.activation(
                        s[:], ta[:], mybir.ActivationFunctionType.Relu,
                        bias=sb_n2[:, gcol : gcol + 1],
                        scale=sb_n2n[:, gcol : gcol + 1],
                    )
                else:
                    s = spool.tile([128, 128], F16, tag="sb")
                    nc.vector.tensor_scalar(
                        s[:], io,
                        sb_dloc[:, gcol : gcol + 1], sb_n2[:, gcol : gcol + 1],
                        mybir.AluOpType.is_equal, mybir.AluOpType.mult,
                    )
                return s

            # ---- main loop: gather groups (gwaves) / compute waves ----
            with loop_cm:
              for g, gtiles in enumerate(plan["gwaves"]):
                  mtiles = {}
                  for cc in range(NCHUNK):
                      nidx = plan["call_nidx"][g][cc]
                      if nidx == 0:
                          continue
                      m = mgp.tile([128, nidx // 128, 128], F16, tag="mg")
                      g16 = plan["call_g16"][g][cc]
                      nc.gpsimd.dma_gather(
                          m[:, : nidx // 128, :],
                          xt[cc * CHUNK : (cc + 1) * CHUNK, :],
                          sb_eidx[:, g16 : g16 + nidx // 16],
                          nidx, nidx, FEAT,
                          single_packet=(nidx <= 1024),
                          queue_num=cc,
                      )
                      mtiles[cc] = m
                  nsw = len(gtiles)
                  ms = msp.tile([128, nsw, 128], F16, tag="ms")
                  r0 = gtiles[0] * 128
                  nc.sync.dma_start(
                      ms[:], xself[r0 : r0 + nsw * 128, :].rearrange("(n p) d -> p n d", p=128)
                  )

                  if mode == "gather":
                      o = fin.tile([128, 128], F32, tag="gonly", name="gonly")
                      nc.vector.tensor_copy(o[:], ms[:, 0, :])
                      nc.sync.dma_start(outT[:, gtiles[0] * 128 : gtiles[0] * 128 + 128], o[:])
                      for cc in range(NCHUNK):
                          if cc in mtiles:
                              o2 = fin.tile([128, 128], F32, tag="gonly2", name="gonly2")
                              nc.vector.tensor_copy(o2[:], mtiles[cc][:, 0, :])
                              nc.sync.dma_start(outT[:, gtiles[0] * 128 : gtiles[0] * 128 + 128], o2[:])
                      continue

                  per_tile = {t: [] for t in gtiles}
                  for cc in range(NCHUNK):
                      if plan["call_nidx"][g][cc] == 0:
                          continue
                      call_base = plan["call_slot"][g][cc]
                      for j, hits in enumerate(plan["chunk_mms"][g][cc]):
                          gcol = (call_base + j * 128) // 128
                          for (t, side) in hits:
                              per_tile[t].append((cc, j, gcol, side))

                  for cw0 in range(0, len(gtiles), VT):
                      cwave = gtiles[cw0 : cw0 + VT]
                      pairs = {}
                      started = {}

                      def pair_half(t, cwave=cwave, pairs=pairs):
                          ti = t - cwave[0]
                          pi = ti // 2
                          if pi not in pairs:
                              pairs[pi] = aggps.tile([128, 256], F32, tag="agg", name="agg")
                          return pairs[pi], (ti % 2) * 128

                      for t in cwave:
                          for (cc, j, gcol, side) in per_tile[t]:
                              ppair, half = pair_half(t)
                              s = build_s(gcol, side)
                              nc.tensor.matmul(
                                  ppair[:, half : half + 128],
                                  mtiles[cc][:, j, :],
                                  s[:],
                                  start=(t not in started), stop=False,
                                  skip_group_check=True,
                              )
                              started[t] = True
                          ppair, half = pair_half(t)
                          tig = t - gtiles[0]
                          gcol = (plan["self_slot"][g] + tig * 128) // 128
                          s = build_s(gcol, 0)
                          nc.tensor.matmul(
                              ppair[:, half : half + 128], ms[:, tig, :], s[:],
                              start=(t not in started), stop=True, skip_group_check=True,
                          )

                      for pi in sorted(pairs):
                          ppair = pairs[pi]
                          asb = fin.tile([128, 256], F32, tag="asb")
                          nc.vector.tensor_copy(asb[:], ppair[:])
                          op = outps.tile([128, 256], F32, tag="op")
                          nc.tensor.matmul(op[:], sb_w[:], asb[:], skip_group_check=True)
                          osb = fin.tile([128, 256], F32, tag="osb")
                          nc.scalar.activation(
                              osb[:], op[:], mybir.ActivationFunctionType.Identity,
                              bias=sb_bcol[:],
                          )
                          base = (cwave[0] + pi * 2) * 128
                          nc.sync.dma_start(outT[:, base : base + 256], osb[:])
    nc.compile()
    return nc


_CACHE = {}


def _get_compiled(src, dst, cnt):
    plan = build_plan(src, dst, cnt)
    key = (plan["total_slots"], plan["B"].tobytes())
    if key not in _CACHE:
        _CACHE[key] = (build_bass(plan), plan)
    else:
        _CACHE[key] = (_CACHE[key][0], plan)
    return _CACHE[key]


def kernel(x, edge_index, W, b):
    from concourse.bass_utils import run_bass_kernel_spmd

    x = np.asarray(x)
    edge_index = np.asarray(edge_index)
    W = np.asarray(W)
    b = np.asarray(b)
    src = edge_index[0].astype(np.int64)
    dst = edge_index[1].astype(np.int64)
    cnt = np.bincount(dst, minlength=N)

    nc, plan = _get_compiled(src, dst, cnt)

    xt = np.zeros((NPAD, FEAT), np.float16)
    xt[:N] = x.astype(np.float16)
    iota = np.tile(np.arange(256, dtype=np.float16), (128, 1))
    wf = W.astype(np.float32)
    bcol = b.astype(np.float32).reshape(FEAT, 1)

    in_maps = []
    for m in range(NCORES):
        in_maps.append({
            "xt": xt,
            "xself": xt[m * PC : (m + 1) * PC],
            "eidx": plan["eidx"][m],
            "dloc": plan["dloc"][m],
            "cs": plan["cs"][m],
            "cd": plan["cd"][m],
            "fac": plan["fac"][m],
            "w": wf,
            "bcol": bcol,
            "iota": iota,
        })
    res = run_bass_kernel_spmd(nc, in_maps, list(range(NCORES)))
    outT = np.concatenate([res.results[m]["outT"] for m in range(NCORES)], axis=1)
    return np.ascontiguousarray(outT[:, :N].T).astype(np.float32)



# revision 5
# speedup vs baseline: 1.9217x; 1.0970x over previous
"""GCN layer (improved self-loops) on 8 Trainium2 NeuronCores.

out = D^{-1/2} (A + 2I) D^{-1/2} X W + b,  deg = in_count + 2.

Strategy (SPMD, one program for all 8 cores; only input data differs per core):
  - Nodes sharded by destination: core m owns rows [m*12544, (m+1)*12544).
  - Aggregation BEFORE the matmul: agg[j] = sum_{e->j} n2_e x[src_e] (+ self),
    out = agg @ W + b, with n2_e = dinv[src] dinv[dst] (2 dinv^2 for self).
  - Per-edge gather of x rows via the custom SWDGE dma_gather instruction
    (int16 indices -> the 100352-row fp16 table is split into 4 chunks), with
    the 4 chunks' gather calls issued on the 4 SWDGE queues so all 8 GpSimd
    Q7 cores generate DMA descriptors concurrently (descriptor generation is
    the kernel bottleneck: ~10 ns/row per queue).
  - Scatter-add via one-hot matmuls on the tensor engine, at PAIR granularity:
    edges bucketed by (dst tile-pair of 256 nodes, src chunk); for each
    128-edge chunk, S[e, d] = (dloc_e == d) * n2_e over d in 0..255 (+256 for
    odd pairs so straddling chunks can't cross-match), PSUM [feat, 256].
  - Position-static structure: bucket capacities = round16(max bucket size
    over the 8 cores), so the single SPMD instruction stream is valid for
    every core; padding slots gather row 0 with n2 = 0.
  - Self loops folded into the PSUM->SBUF copy: host uploads
    selfT[f, i] = 2 dinv_i^2 x[i, f] (transposed, prescaled) and the copy is
    a DVE tensor_tensor add.  Normalization n2 is precomputed on host (fp16).
  - Final per-pair matmul with W in fp16, bias via ACT, fp16 output stored
    transposed [128 feat, nodes]; host transposes back.
"""

import sys

sys.path.insert(0, "/opt/trn_rl_repo")

import numpy as np

import concourse.bacc as bacc
import concourse.mybir as mybir
import concourse.tile as tile

F32 = mybir.dt.float32
F16 = mybir.dt.float16
I16 = mybir.dt.int16

N = 100000
FEAT = 128
NCORES = 8
PC = 12544            # nodes per core
NPAD = PC * NCORES    # 100352
TILES = PC // 128     # 98
PAIRS = TILES // 2    # 49
NCHUNK = 4
CHUNK = NPAD // NCHUNK  # 25088 rows per gather chunk
GP = 10               # pairs per gather wave (5 waves: 10,10,10,10,9)
ACT_EVERY = 5         # every ACT_EVERY'th S-build goes to the scalar engine


def _round128(x):
    return int(-(-int(x) // 128) * 128)


def build_plan(src, dst, cnt):
    """Host-side integer metadata. src/dst: int64 [E]; cnt: int64 [N] in-degree.

    Buckets are per (dst pair of 256 nodes, src chunk); capacities are the max
    bucket size over the 8 cores rounded to 16. 128-edge matmul chunks may
    straddle two adjacent pair-buckets; pair parity is encoded into dloc
    (+256 for odd pairs) so the two matmuls of a straddling chunk each only
    match their own pair's edges.
    """
    E = src.shape[0]
    core = dst // PC
    dl = dst - core * PC          # 0..PC-1
    p = dl >> 8                   # pair in core (0..48)
    c = src // CHUNK              # source chunk
    flat = (core * PAIRS + p) * NCHUNK + c
    bc = np.bincount(flat, minlength=NCORES * PAIRS * NCHUNK).reshape(
        NCORES, PAIRS, NCHUNK
    )
    B = bc.max(axis=0)            # [PAIRS, NCHUNK]
    B = ((B + 15) // 16) * 16     # capacities, multiple of 16

    gwaves = [list(range(g * GP, min((g + 1) * GP, PAIRS)))
              for g in range((PAIRS + GP - 1) // GP)]

    # --- slot layout (same for every core) ---
    bucket_base = np.zeros((PAIRS, NCHUNK), np.int64)  # global slot base
    call_nidx = []   # [wave][chunk] -> num idxs (mult of 128)
    call_slot = []   # [wave][chunk] -> slot base
    chunk_mms = []   # [wave][chunk] -> list per 128-chunk of [pair,...]
    pos = 0
    for wave in gwaves:
        nidx_w, slot_w, mm_w = [], [], []
        for cc in range(NCHUNK):
            nonempty = [pp for pp in wave if B[pp, cc] > 0]
            raw = int(sum(B[pp, cc] for pp in nonempty))
            nidx = _round128(raw)
            slot_w.append(pos)
            nidx_w.append(nidx)
            spans = []
            off = 0
            for pp in nonempty:
                bucket_base[pp, cc] = pos + off
                spans.append((off, off + int(bc[:, pp, cc].max()), pp))
                off += int(B[pp, cc])
            mms = []
            for j in range(nidx // 128):
                lo, hi = j * 128, (j + 1) * 128
                hit = [pp for (s0, s1, pp) in spans if not (s1 <= lo or s0 >= hi)]
                assert len(hit) <= 2, (len(hit), j, cc)
                mms.append(hit)
            mm_w.append(mms)
            pos += nidx
        call_nidx.append(nidx_w)
        call_slot.append(slot_w)
        chunk_mms.append(mm_w)
    total_slots = pos
    cols = total_slots // 128
    gcols16 = total_slots // 16

    # --- per-core arrays ---
    cnt_pad = np.zeros(NPAD, np.int64)
    cnt_pad[:N] = cnt
    dinv_pad = np.zeros(NPAD, np.float64)
    dinv_pad[:N] = 1.0 / np.sqrt(cnt + 2.0)

    eidx_flat = np.zeros((NCORES, total_slots), np.int16)
    dloc_flat = np.zeros((NCORES, total_slots), np.float16)
    n2_flat = np.zeros((NCORES, total_slots), np.float16)

    order_all = np.argsort(core * (PAIRS * NCHUNK) + p * NCHUNK + c, kind="stable")
    flat_sorted = flat[order_all]
    starts = np.searchsorted(flat_sorted, np.arange(NCORES * PAIRS * NCHUNK),
                             side="left")
    rank = np.arange(E) - starts[flat_sorted]
    bb_flat = np.broadcast_to(bucket_base, (NCORES, PAIRS, NCHUNK)).reshape(-1)
    slots_sorted = bb_flat[flat_sorted] + rank
    cores_sorted = core[order_all]
    src_sorted = src[order_all]
    dst_sorted = dst[order_all]
    c_sorted = c[order_all]
    p_sorted = p[order_all]
    n2_all = (dinv_pad[src_sorted] * dinv_pad[dst_sorted]).astype(np.float16)
    for m in range(NCORES):
        sel = cores_sorted == m
        sl = slots_sorted[sel]
        eidx_flat[m, sl] = (src_sorted[sel] - c_sorted[sel] * CHUNK).astype(np.int16)
        dloc_flat[m, sl] = ((dst_sorted[sel] & 255) + 256 * (p_sorted[sel] & 1)
                            ).astype(np.float16)
        n2_flat[m, sl] = n2_all[sel]

    def wrap(a):
        return np.ascontiguousarray(a.reshape(-1, 128).T)

    eidx = np.zeros((NCORES, 128, gcols16), np.int16)
    for m in range(NCORES):
        w16 = eidx_flat[m].reshape(-1, 16).T
        eidx[m] = np.tile(w16, (8, 1))

    return dict(
        B=B, gwaves=gwaves, call_nidx=call_nidx, call_slot=call_slot,
        chunk_mms=chunk_mms, total_slots=total_slots, gcols16=gcols16, cols=cols,
        eidx=eidx,
        dloc=np.stack([wrap(dloc_flat[m]) for m in range(NCORES)]),
        n2=np.stack([wrap(n2_flat[m]) for m in range(NCORES)]),
        dinv_pad=dinv_pad,
    )


def build_bass(plan, repeat=1):
    """Build the SPMD Bass program for the static structure in `plan`."""
    gwaves = plan["gwaves"]
    cols = plan["cols"]
    gcols16 = plan["gcols16"]

    nc = bacc.Bacc(
        "TRN2", target_bir_lowering=False, debug=False, num_swdge_queues=4
    )
    xt = nc.dram_tensor("xt", [NPAD, FEAT], F16, kind="ExternalInput")
    selfT_d = nc.dram_tensor("selfT", [FEAT, PC], F16, kind="ExternalInput")
    eidx_d = nc.dram_tensor("eidx", [128, gcols16], I16, kind="ExternalInput")
    dloc_d = nc.dram_tensor("dloc", [128, cols], F16, kind="ExternalInput")
    n2_d = nc.dram_tensor("n2", [128, cols], F16, kind="ExternalInput")
    w_d = nc.dram_tensor("w", [FEAT, FEAT], F16, kind="ExternalInput")
    bcol_d = nc.dram_tensor("bcol", [FEAT, 1], F32, kind="ExternalInput")
    iota_d = nc.dram_tensor("iota", [128, 512], F16, kind="ExternalInput")
    outT = nc.dram_tensor("outT", [FEAT, PC], F16, kind="ExternalOutput")

    with tile.TileContext(nc) as tc:
        with (
            tc.tile_pool(name="meta", bufs=1) as meta,
            tc.tile_pool(name="mg", bufs=2) as mgp,
            tc.tile_pool(name="sld", bufs=3) as sldp,
            tc.tile_pool(name="sp", bufs=16) as spool,
            tc.tile_pool(name="fin", bufs=6) as fin,
            tc.tile_pool(name="aggps", bufs=6, space="PSUM") as aggps,
            tc.tile_pool(name="outps", bufs=2, space="PSUM") as outps,
        ):
            # ---- prologue: eidx first (gates the gathers), then the rest ----
            sb_eidx = meta.tile([128, gcols16], I16, tag="eidx")
            nc.sync.dma_start(sb_eidx[:], eidx_d[:])
            sb_dloch = meta.tile([128, cols], F16, tag="dloch")
            nc.sync.dma_start(sb_dloch[:], dloc_d[:])
            sb_n2h = meta.tile([128, cols], F16, tag="n2h")
            nc.sync.dma_start(sb_n2h[:], n2_d[:])
            sb_w = meta.tile([FEAT, FEAT], F16, tag="w")
            nc.sync.dma_start(sb_w[:], w_d[:])
            sb_bcol = meta.tile([FEAT, 1], F32, tag="bcol")
            nc.sync.dma_start(sb_bcol[:], bcol_d[:])
            sb_iota = meta.tile([128, 512], F16, tag="iota")
            nc.sync.dma_start(sb_iota[:], iota_d[:])

            sb_dloc = meta.tile([128, cols], F32, tag="dlocf")
            nc.vector.tensor_copy(sb_dloc[:], sb_dloch[:])
            sb_n2 = meta.tile([128, cols], F32, tag="n2f")
            nc.vector.tensor_copy(sb_n2[:], sb_n2h[:])
            sb_n2n = meta.tile([128, cols], F32, tag="n2n")
            nc.vector.tensor_scalar_mul(sb_n2n[:], sb_n2[:], -1.0)

            sb_count = [0]
            import contextlib
            loop_cm = tc.For_i(0, repeat, 1) if repeat > 1 else contextlib.nullcontext()

            def build_s(gcol, side):
                """S[e, d] = (dloc[e] == side*256 + d) * n2[e] for the
                128-edge chunk at global column gcol. Alternates DVE / ACT."""
                sb_count[0] += 1
                io = sb_iota[:, side * 256 : side * 256 + 256]
                if sb_count[0] % ACT_EVERY == 0:
                    ta = spool.tile([128, 256], F16, tag="sa")
                    nc.scalar.activation(
                        ta[:], io, mybir.ActivationFunctionType.Abs,
                        bias=sb_dloc[:, gcol : gcol + 1], scale=-1.0,
                    )
                    s = spool.tile([128, 256], F16, tag="sb")
                    nc.scalar.activation(
                        s[:], ta[:], mybir.ActivationFunctionType.Relu,
                        bias=sb_n2[:, gcol : gcol + 1],
                        scale=sb_n2n[:, gcol : gcol + 1],
                    )
                else:
                    s = spool.tile([128, 256], F16, tag="sb")
                    nc.vector.tensor_scalar(
                        s[:], io,
                        sb_dloc[:, gcol : gcol + 1], sb_n2[:, gcol : gcol + 1],
                        mybir.AluOpType.is_equal, mybir.AluOpType.mult,
                    )
                return s

            # ---- main loop over gather waves ----
            with loop_cm:
              for g, gpairs in enumerate(gwaves):
                  mtiles = {}
                  for cc in range(NCHUNK):
                      nidx = plan["call_nidx"][g][cc]
                      if nidx == 0:
                          continue
                      m = mgp.tile([128, nidx // 128, 128], F16, tag=f"mg{cc}")
                      g16 = plan["call_slot"][g][cc] // 16
                      nc.gpsimd.dma_gather(
                          m[:, : nidx // 128, :],
                          xt[cc * CHUNK : (cc + 1) * CHUNK, :],
                          sb_eidx[:, g16 : g16 + nidx // 16],
                          nidx, nidx, FEAT,
                          single_packet=(nidx <= 1024),
                          queue_num=cc,
                      )
                      mtiles[cc] = m

                  npair = len(gpairs)
                  sld = sldp.tile([128, npair * 256], F16, tag="sld")
                  nc.sync.dma_start(
                      sld[:], selfT_d[:, gpairs[0] * 256 : gpairs[0] * 256 + npair * 256]
                  )

                  per_pair = {pp: [] for pp in gpairs}
                  for cc in range(NCHUNK):
                      if plan["call_nidx"][g][cc] == 0:
                          continue
                      call_base = plan["call_slot"][g][cc]
                      for j, hits in enumerate(plan["chunk_mms"][g][cc]):
                          gcol = (call_base + j * 128) // 128
                          for pp in hits:
                              per_pair[pp].append((cc, j, gcol))

                  for pp in gpairs:
                      mms = per_pair[pp]
                      side = pp & 1
                      ppsum = aggps.tile([128, 256], F32, tag="agg", name="agg")
                      for k, (cc, j, gcol) in enumerate(mms):
                          s = build_s(gcol, side)
                          nc.tensor.matmul(
                              ppsum[:], mtiles[cc][:, j, :], s[:],
                              start=(k == 0), stop=(k == len(mms) - 1),
                              skip_group_check=True,
                          )
                      pi = pp - gpairs[0]
                      asb = fin.tile([128, 256], F16, tag="asb")
                      nc.vector.tensor_tensor(
                          asb[:], ppsum[:], sld[:, pi * 256 : pi * 256 + 256],
                          mybir.AluOpType.add,
                      )
                      op = outps.tile([128, 256], F32, tag="op")
                      nc.tensor.matmul(op[:], sb_w[:], asb[:], skip_group_check=True)
                      osb = fin.tile([128, 256], F16, tag="osb")
                      nc.scalar.activation(
                          osb[:], op[:], mybir.ActivationFunctionType.Identity,
                          bias=sb_bcol[:],
                      )
                      nc.sync.dma_start(outT[:, pp * 256 : pp * 256 + 256], osb[:])
    nc.compile()
    return nc


_CACHE = {}


def _get_compiled(src, dst, cnt):
    plan = build_plan(src, dst, cnt)
    key = (plan["total_slots"], plan["B"].tobytes())
    if key not in _CACHE:
        _CACHE[key] = (build_bass(plan), plan)
    else:
        _CACHE[key] = (_CACHE[key][0], plan)
    return _CACHE[key]


def kernel(x, edge_index, W, b):
    from concourse.bass_utils import run_bass_kernel_spmd

    x = np.asarray(x)
    edge_index = np.asarray(edge_index)
    W = np.asarray(W)
    b = np.asarray(b)
    src = edge_index[0].astype(np.int64)
    dst = edge_index[1].astype(np.int64)
    cnt = np.bincount(dst, minlength=N)

    nc, plan = _get_compiled(src, dst, cnt)

    xt = np.zeros((NPAD, FEAT), np.float16)
    xt[:N] = x.astype(np.float16)
    iota = np.tile(np.arange(512, dtype=np.float16), (128, 1))
    wf = W.astype(np.float16)
    bcol = b.astype(np.float32).reshape(FEAT, 1)

    # selfT[f, i] = 2 * dinv_i^2 * x[i, f] per core, transposed + prescaled
    dinv = plan["dinv_pad"]  # [NPAD], 0 for pad rows
    in_maps = []
    for m in range(NCORES):
        lo, hi = m * PC, (m + 1) * PC
        w_self = 2.0 * dinv[lo:hi] ** 2
        xs = np.zeros((PC, FEAT), np.float64)
        real = min(hi, N) - lo
        if real > 0:
            xs[:real] = x[lo : lo + real].astype(np.float64) * w_self[:real, None]
        in_maps.append({
            "xt": xt,
            "selfT": np.ascontiguousarray(xs.T).astype(np.float16),
            "eidx": plan["eidx"][m],
            "dloc": plan["dloc"][m],
            "n2": plan["n2"][m],
            "w": wf,
            "bcol": bcol,
            "iota": iota,
        })
    res = run_bass_kernel_spmd(nc, in_maps, list(range(NCORES)))
    outT = np.concatenate([res.results[m]["outT"] for m in range(NCORES)], axis=1)
    return np.ascontiguousarray(outT[:, :N].T).astype(np.float32)
